# revision 37
# baseline (speedup 1.0000x reference)
"""ConViT (12-layer, H=12, D=64, B=64) forward pass on 8 TRN2 NeuronCores.

Strategy: data-parallel over batch (8 images per core). Host does layout prep
only (im2col of the non-overlapping patch conv, weight transposes to [ci, co],
bf16 casts); all FLOPs run on-device. Activations are feature-major
[C -> 6x128 partitions, tokens free]; matmuls are bf16 with f32 PSUM;
residual stream / layernorm / softmax arithmetic in f32.
"""
import os
import sys
import types
import contextlib
import ctypes
from contextlib import ExitStack

import numpy as np
import ml_dtypes

import concourse.bass as bass
import concourse.mybir as mybir
import concourse.tile as tile
from concourse import bacc
from concourse.masks import make_identity

F32 = mybir.dt.float32
BF16 = mybir.dt.bfloat16
FP16 = mybir.dt.float16
AF = mybir.ActivationFunctionType
ALU = mybir.AluOpType
BF = ml_dtypes.bfloat16

H = 12
D = 64
C = 768
NCH = 6             # C / 128
PS = 16             # patch size
GRID = 14
NPATCH = 196        # tokens per image in GPSA phase
NTOK = 197          # tokens per image in MHSA phase (cls + patches)
L_G = 3
L_M = 9
SCALE = D ** -0.5
EPS = 1e-5
B_CORE = 8
NCORES = 8
MLPC = 24           # 3072 / 128

_CACHE = {}


def _install_ntff_hook():
    """Best-effort: enable NTFF profiling under axon (used when BASS_TRACE=1)."""
    if "antenv.axon_hooks" in sys.modules:
        return
    so_path = "/opt/axon/libaxon_pjrt.so"
    if not os.path.exists(so_path):
        return
    try:
        lib = ctypes.CDLL(so_path)
        if not hasattr(lib, "axon_start_nrt_profile"):
            return
        lib.axon_start_nrt_profile.argtypes = [ctypes.POINTER(ctypes.c_int64), ctypes.c_size_t]
        lib.axon_start_nrt_profile.restype = ctypes.c_int64
        lib.axon_stop_nrt_profile.argtypes = [ctypes.c_char_p]
        lib.axon_stop_nrt_profile.restype = ctypes.c_int64

        @contextlib.contextmanager
        def _hook(output_dir, device_ids):
            import jax
            jax.devices()
            if device_ids:
                ids = (ctypes.c_int64 * len(device_ids))(*device_ids)
                rc = lib.axon_start_nrt_profile(ids, len(device_ids))
            else:
                rc = lib.axon_start_nrt_profile(None, 0)
            if rc != 0:
                raise RuntimeError(f"axon_start_nrt_profile rc={rc}")
            try:
                yield
            finally:
                n = lib.axon_stop_nrt_profile(str(output_dir).encode())
                if n < 0:
                    raise RuntimeError(f"axon_stop_nrt_profile rc={n}")

        mod = types.ModuleType("antenv.axon_hooks")
        mod._hook = _hook
        mod.get_axon_ntff_profile_hook = lambda: mod._hook
        mod.set_axon_ntff_profile_hook = lambda h: setattr(mod, "_hook", h)
        sys.modules["antenv.axon_hooks"] = mod
        import antenv
        antenv.axon_hooks = mod
    except Exception:
        pass


def _build_program():
    nc = bacc.Bacc("TRN2", target_bir_lowering=False, debug=False)

    def din(name, shape, dt):
        return nc.dram_tensor(name, shape, dt, kind="ExternalInput").ap()

    xim = din("xim", [C, B_CORE * NPATCH], BF16)
    pwT = din("pwT", [C, C], BF16)
    pb = din("pb", [128, NCH], F32)
    posT = din("posT", [C, NPATCH], F32)
    cls = din("cls", [C], F32)
    rel = din("rel", [NPATCH, 3, NPATCH], FP16)
    headT = din("headT", [C, 1024], BF16)
    head_b = din("head_b", [128, 8], F32)
    fns = din("fns", [128, NCH], F32)
    fnb = din("fnb", [128, NCH], F32)

    gl, ml = [], []
    for i in range(L_G):
        gl.append({
            "qkT": din(f"g{i}_qkT", [C, 2 * C], BF16),
            "vT": din(f"g{i}_vT", [C, C], BF16),
            "projT": din(f"g{i}_projT", [C, C], BF16),
            "projb": din(f"g{i}_projb", [128, NCH], F32),
            "fc1T": din(f"g{i}_fc1T", [C, 4 * C], BF16),
            "fc1b": din(f"g{i}_fc1b", [128, MLPC], F32),
            "fc2T": din(f"g{i}_fc2T", [4 * C, C], BF16),
            "fc2b": din(f"g{i}_fc2b", [128, NCH], F32),
            "n1s": din(f"g{i}_n1s", [128, NCH], F32),
            "n1b": din(f"g{i}_n1b", [128, NCH], F32),
            "n2s": din(f"g{i}_n2s", [128, NCH], F32),
            "n2b": din(f"g{i}_n2b", [128, NCH], F32),
            "posw": din(f"g{i}_posw", [H, 3], F32),
            "gate": din(f"g{i}_gate", [1, H], F32),
        })
    for i in range(L_M):
        ml.append({
            "qkvT": din(f"m{i}_qkvT", [C, 3 * C], BF16),
            "projT": din(f"m{i}_projT", [C, C], BF16),
            "projb": din(f"m{i}_projb", [128, NCH], F32),
            "fc1T": din(f"m{i}_fc1T", [C, 4 * C], BF16),
            "fc1b": din(f"m{i}_fc1b", [128, MLPC], F32),
            "fc2T": din(f"m{i}_fc2T", [4 * C, C], BF16),
            "fc2b": din(f"m{i}_fc2b", [128, NCH], F32),
            "n1s": din(f"m{i}_n1s", [128, NCH], F32),
            "n1b": din(f"m{i}_n1b", [128, NCH], F32),
            "n2s": din(f"m{i}_n2s", [128, NCH], F32),
            "n2b": din(f"m{i}_n2b", [128, NCH], F32),
        })

    OUT = nc.dram_tensor("out", [1024, B_CORE], F32, kind="ExternalOutput").ap()

    MIDTAGS = [f"qt{j}" for j in range(NCH)] + [f"kt{j}" for j in range(NCH)]

    with ExitStack() as ctx:
        tc = ctx.enter_context(tile.TileContext(nc))

        consts = ctx.enter_context(tc.tile_pool(name="consts", bufs=1))
        res_p = ctx.enter_context(tc.tile_pool(name="res", bufs=1))
        act_p = ctx.enter_context(tc.tile_pool(name="act", bufs=2))   # xn / OT / xn2
        qk_p = ctx.enter_context(tc.tile_pool(name="qk", bufs=2))     # Q^T/K^T + MLP mids
        v_p = ctx.enter_context(tc.tile_pool(name="vp", bufs=1))      # token-major V
        w_p = ctx.enter_context(tc.tile_pool(name="wp", bufs=1))      # streamed weights
        wsm_p = ctx.enter_context(tc.tile_pool(name="wsm", bufs=1))   # per-layer params
        row_p = ctx.enter_context(tc.tile_pool(name="rows", bufs=1))  # [1, *] stat rows
        rowa_p = ctx.enter_context(tc.tile_pool(name="rowsa", bufs=1))  # attn recip rows
        tmp_p = ctx.enter_context(tc.tile_pool(name="tmp", bufs=2))   # f32 scratch tiles
        tmp1_p = ctx.enter_context(tc.tile_pool(name="tmp1", bufs=1))  # big f32 scratch
        e_p = ctx.enter_context(tc.tile_pool(name="ep", bufs=2))      # exp tiles
        xq_p = ctx.enter_context(tc.tile_pool(name="xqp", bufs=2))    # ln squares
        ft_p = ctx.enter_context(tc.tile_pool(name="ft", bufs=1))     # GPSA pos F^T

        ones_col = consts.tile([128, 1], BF16)
        nc.vector.memset(ones_col, 1.0)
        ones_row = consts.tile([1, 128], BF16)
        nc.vector.memset(ones_row, 1.0)
        ident = consts.tile([128, 128], BF16)
        make_identity(nc, ident)
        ones128 = consts.tile([128, 128], BF16)
        nc.vector.memset(ones128, 1.0)
        eps_sb = consts.tile([128, 1], F32)
        nc.vector.memset(eps_sb, EPS)

        res = [res_p.tile([128, B_CORE, NTOK], F32, tag=f"res{c}", name=f"res{c}") for c in range(NCH)]

        rel_sb = [consts.tile([128, 3, NPATCH], FP16, tag=f"rel{i}", name=f"rel{i}") for i in range(2)]
        nc.sync.dma_start(out=rel_sb[0][:128], in_=rel[0:128])
        nc.sync.dma_start(out=rel_sb[1][:68], in_=rel[128:196])

        # cls token into res[:, :, 0]
        for c in range(NCH):
            src = cls[c * 128:(c + 1) * 128]
            ap = bass.AP(tensor=src.tensor, offset=src.offset,
                         ap=[list(src.ap[0]), [0, B_CORE], [0, 1]])
            nc.sync.dma_start(out=res[c][:, :, 0:1], in_=ap)

        def load_wT(dram, ncol, tag):
            t = w_p.tile([128, NCH, ncol], BF16, tag=tag)
            nc.sync.dma_start(out=t, in_=dram.rearrange("(c p) n -> p c n", p=128))
            return t

        def load_sm(dram, ncol, tag, dt=F32):
            t = wsm_p.tile([128, ncol], dt, tag=tag)
            nc.sync.dma_start(out=t, in_=dram)
            return t

        # ---- patch embed -------------------------------------------------
        pw_sb = load_wT(pwT, C, "wbig")
        pb_sb = load_sm(pb, NCH, "pb")
        xim_sb = w_p.tile([128, 13, C], BF16, tag="wbig2")
        ximv = xim_sb.rearrange("p a b -> p (a b)")[:, 0:NCH * B_CORE * NPATCH] \
            .rearrange("p (c n) -> p c n", c=NCH)
        nc.sync.dma_start(out=ximv, in_=xim.rearrange("(c p) n -> p c n", p=128))

        with tc.tile_pool(name="pe_ps", bufs=4, space="PSUM") as pps:
            for nv in range(4):
                b0 = 2 * nv
                for mc in range(NCH):
                    ps = pps.tile([128, 2 * NPATCH], F32, tag="mm")
                    for kc in range(NCH):
                        nc.tensor.matmul(
                            ps, pw_sb[:, kc, mc * 128:(mc + 1) * 128],
                            ximv[:, kc, b0 * NPATCH:(b0 + 2) * NPATCH],
                            start=(kc == 0), stop=(kc == NCH - 1))
                    nc.scalar.activation(
                        out=res[mc][:, b0:b0 + 2, 1:NTOK],
                        in_=ps.rearrange("p (b t) -> p b t", b=2),
                        func=AF.Identity, bias=pb_sb[:, mc:mc + 1], scale=1.0)
        for c in range(NCH):
            src = posT[c * 128:(c + 1) * 128]
            ap = bass.AP(tensor=src.tensor, offset=src.offset,
                         ap=[list(src.ap[0]), [0, B_CORE], list(src.ap[1])])
            nc.gpsimd.dma_start(out=res[c][:, :, 1:NTOK], in_=ap, accum_op=ALU.add)

        # ---- helpers -----------------------------------------------------
        def ln(xn, svt, bvt, t0, tl, st_pool, mm_pool):
            """res -> xn bf16 normalized; tokens [t0, t0+tl) of each image.

            Software-pipelined: group nv+1's stats are issued before group
            nv's rows/broadcast/epilogue so the PE never waits a full
            vector chain between stat matmuls."""
            ntl = 2 * tl

            def ln_stats(nv):
                b0 = 2 * nv
                s_ps = st_pool.tile([1, ntl], F32, tag="st1")
                q_ps = st_pool.tile([1, ntl], F32, tag="st2")
                for c in range(NCH):
                    xb = tmp_p.tile([128, 2, tl], BF16, tag="xb")
                    xq = xq_p.tile([128, 2, tl], BF16, tag="xq")
                    sl = res[c][:, b0:b0 + 2, t0:t0 + tl]
                    nc.vector.tensor_copy(out=xb, in_=sl)
                    xbf = xb.rearrange("p b t -> p (b t)")
                    xqf = xq.rearrange("p b t -> p (b t)")
                    nc.vector.tensor_tensor(out=xqf, in0=xbf, in1=xbf, op=ALU.mult)
                    nc.tensor.matmul(s_ps, ones_col, xbf,
                                     start=(c == 0), stop=(c == NCH - 1))
                    nc.tensor.matmul(q_ps, ones_col, xqf,
                                     start=(c == 0), stop=(c == NCH - 1))
                return s_ps, q_ps

            def ln_finish(nv, s_ps, q_ps):
                b0 = 2 * nv
                mean = row_p.tile([1, ntl], F32, tag="r1")
                var = row_p.tile([1, ntl], F32, tag="r2")
                msq = row_p.tile([1, ntl], F32, tag="r3")
                nc.vector.tensor_scalar_mul(mean, s_ps, 1.0 / C)
                nc.vector.tensor_scalar_mul(var, q_ps, 1.0 / C)
                nc.vector.tensor_tensor(out=msq, in0=mean, in1=mean, op=ALU.mult)
                nc.vector.tensor_sub(var, var, msq)
                nc.scalar.activation(out=var, in_=var, func=AF.Sqrt,
                                     bias=eps_sb[0:1, :], scale=1.0)
                nc.vector.reciprocal_approx_fast(out=msq, in_=var)
                meanb = row_p.tile([1, ntl], BF16, tag="r4")
                rstdb = row_p.tile([1, ntl], BF16, tag="r5")
                nc.vector.tensor_copy(out=meanb, in_=mean)
                nc.vector.tensor_copy(out=rstdb, in_=msq)
                mR = mm_pool.tile([128, ntl], F32, tag="mm")
                rR = mm_pool.tile([128, ntl], F32, tag="mm")
                nc.tensor.matmul(mR, ones_row, meanb, start=True, stop=True)
                nc.tensor.matmul(rR, ones_row, rstdb, start=True, stop=True)
                mRv = mR.rearrange("p (b t) -> p b t", b=2)
                rRv = rR.rearrange("p (b t) -> p b t", b=2)
                for c in range(NCH):
                    sl = res[c][:, b0:b0 + 2, t0:t0 + tl]
                    t = tmp1_p.tile([128, 2, tl], F32, tag="lt")
                    tb = tmp1_p.tile([128, 2, tl], BF16, tag="ltb")
                    nc.vector.tensor_sub(t, sl, mRv)
                    nc.vector.scalar_tensor_tensor(
                        out=tb, in0=t, scalar=svt[:, c:c + 1], in1=rRv,
                        op0=ALU.mult, op1=ALU.mult)
                    nc.vector.tensor_scalar(
                        out=xn[c][:, b0:b0 + 2, t0:t0 + tl], in0=tb,
                        scalar1=bvt[:, c:c + 1], scalar2=None,
                        op0=ALU.add)

            pend = []
            for nv in range(4):
                pend.append(ln_stats(nv))
                if nv > 0:
                    ln_finish(nv - 1, *pend[nv - 1])
            ln_finish(3, *pend[3])

        def add_linear(wT_sb, biast, t0, tl, rhs_of, mm_pool):
            """res += (rhs @ W^T);  then res += bias (only token cols [t0,t0+tl))."""
            for nv in range(4):
                b0 = 2 * nv
                for mc in range(NCH):
                    ps = mm_pool.tile([128, 2 * tl], F32, tag="mm")
                    for kc in range(NCH):
                        nc.tensor.matmul(
                            ps, wT_sb[:, kc, mc * 128:(mc + 1) * 128], rhs_of(kc, b0),
                            start=(kc == 0), stop=(kc == NCH - 1))
                    sl = res[mc][:, b0:b0 + 2, t0:t0 + tl]
                    nc.vector.tensor_tensor(
                        out=sl, in0=sl, in1=ps.rearrange("p (b t) -> p b t", b=2),
                        op=ALU.add)
            for mc in range(NCH):
                sl = res[mc][:, :, t0:t0 + tl]
                nc.vector.tensor_scalar(out=sl, in0=sl, scalar1=biast[:, mc:mc + 1],
                                        scalar2=None, op0=ALU.add)

        def mlp(L, xn, t0, tl, st_pool, mm_pool):
            fc1_sb = load_wT(L["fc1T"], 4 * C, "wbig")
            fc1b_sb = load_sm(L["fc1b"], MLPC, "fc1b")
            fc2_sb = w_p.tile([128, MLPC, C], BF16, tag="wbig2")
            nc.sync.dma_start(out=fc2_sb, in_=L["fc2T"].rearrange("(c p) n -> p c n", p=128))
            fc2b_sb = load_sm(L["fc2b"], NCH, "fc2b")
            for nv in range(4):
                b0 = 2 * nv
                mids = []
                for mc in range(MLPC):
                    ps = mm_pool.tile([128, 2 * tl], F32, tag="mm")
                    for kc in range(NCH):
                        nc.tensor.matmul(
                            ps, fc1_sb[:, kc, mc * 128:(mc + 1) * 128],
                            xn[kc][:, b0:b0 + 2, t0:t0 + tl],
                            start=(kc == 0), stop=(kc == NCH - 1))
                    mt = qk_p.tile([128, 2 * NTOK], BF16, tag=MIDTAGS[mc % 12])
                    nc.scalar.activation(out=mt[:, 0:2 * tl], in_=ps, func=AF.Gelu,
                                         bias=fc1b_sb[:, mc:mc + 1], scale=1.0)
                    mids.append(mt)
                for mc in range(NCH):
                    ps = mm_pool.tile([128, 2 * tl], F32, tag="mm")
                    for kc in range(MLPC):
                        nc.tensor.matmul(
                            ps, fc2_sb[:, kc, mc * 128:(mc + 1) * 128],
                            mids[kc][:, 0:2 * tl],
                            start=(kc == 0), stop=(kc == MLPC - 1))
                    sl = res[mc][:, b0:b0 + 2, t0:t0 + tl]
                    nc.vector.tensor_tensor(
                        out=sl, in0=sl, in1=ps.rearrange("p (b t) -> p b t", b=2),
                        op=ALU.add)
            for mc in range(NCH):
                sl = res[mc][:, :, t0:t0 + tl]
                nc.vector.tensor_scalar(out=sl, in0=sl, scalar1=fc2b_sb[:, mc:mc + 1],
                                        scalar2=None, op0=ALU.add)

        def qkv_and_attention(L, xn, OT, t0, tl, gpsa, pos_ctx):
            """Fused per-image-pair: QKV projections -> attention -> OT."""
            nkeys = [(0, 128), (128, tl - 128)]
            with tc.tile_pool(name=f"qa_mm{id(L)}", bufs=1, space="PSUM") as mmp, \
                 tc.tile_pool(name=f"qa_v{id(L)}", bufs=1, space="PSUM") as vps, \
                 tc.tile_pool(name=f"qa_atA{id(L)}", bufs=(1 if gpsa else 2), space="PSUM") as apsA, \
                 tc.tile_pool(name=f"qa_at{id(L)}", bufs=1, space="PSUM") as aps:
                w_qk = pos_ctx["w_qk"]
                w_v = pos_ctx["w_v"]
                for nv in range(4):
                    b0 = 2 * nv
                    qt = [qk_p.tile([128, 2, NTOK], BF16, tag=f"qt{c}", name=f"qtt{c}") for c in range(NCH)]
                    kt = [qk_p.tile([128, 2, NTOK], BF16, tag=f"kt{c}", name=f"ktt{c}") for c in range(NCH)]
                    for mc in range(2 * NCH):
                        ps = mmp.tile([128, 2 * tl], F32, tag="mm")
                        for kc in range(NCH):
                            nc.tensor.matmul(
                                ps, w_qk[:, kc, mc * 128:(mc + 1) * 128],
                                xn[kc][:, b0:b0 + 2, t0:t0 + tl],
                                start=(kc == 0), stop=(kc == NCH - 1))
                        dst = qt[mc] if mc < NCH else kt[mc - NCH]
                        nc.scalar.activation(
                            out=dst[:, :, 0:tl],
                            in_=ps.rearrange("p (b t) -> p b t", b=2), func=AF.Copy)
                    vt = {}
                    for bi in range(2):
                        b = b0 + bi
                        for hi, (h0, hl) in enumerate(nkeys):
                            vsb = v_p.tile([128, H, 65], BF16, tag=f"v{bi}{hi}")
                            ps = vps.tile([128, C], F32, tag="mmv")
                            for kc in range(NCH):
                                for c0, cl in ((0, 512), (512, 256)):
                                    nc.tensor.matmul(
                                        ps[:hl, c0:c0 + cl],
                                        xn[kc][:, b, t0 + h0:t0 + h0 + hl],
                                        w_v(kc)[:, c0:c0 + cl],
                                        start=(kc == 0), stop=(kc == NCH - 1))
                            nc.vector.tensor_copy(
                                out=vsb[:hl, :, 0:64],
                                in_=ps[:hl].rearrange("p (h d) -> p h d", h=H))
                            nc.vector.memset(vsb[:hl, :, 64:65], 1.0)
                            vt[(bi, hi)] = vsb
                    for bi in range(2):
                        b = b0 + bi
                        kl = tl - 128
                        for ch in range(NCH):
                            opsT = []
                            for j in range(2):
                                h = 2 * ch + j
                                off = j * 64
                                s1 = apsA.tile([128, 2, NTOK], F32, tag="s")
                                s2 = s1[:, 1, :]
                                nc.tensor.matmul(s1[:, 0, 0:tl],
                                                 kt[ch][off:off + 64, bi, 0:128],
                                                 qt[ch][off:off + 64, bi, 0:tl],
                                                 start=True, stop=True)
                                nc.tensor.matmul(s2[:kl, 0:tl],
                                                 kt[ch][off:off + 64, bi, 128:tl],
                                                 qt[ch][off:off + 64, bi, 0:tl],
                                                 start=True, stop=True)
                                ep = e_p.tile([128, 2, NTOK], BF16, tag=f"e{j}")
                                nc.scalar.activation(out=ep[:, :, 0:tl], in_=s1[:, :, 0:tl],
                                                     func=AF.Exp, scale=SCALE)
                                ops = aps.tile([65, NTOK], F32, tag=f"ops{j}")
                                nc.tensor.matmul(ops[:, 0:tl], vt[(bi, 0)][:, h, :],
                                                 ep[:, 0, 0:tl], start=True, stop=False)
                                nc.tensor.matmul(ops[:, 0:tl], vt[(bi, 1)][:kl, h, :],
                                                 ep[:kl, 1, 0:tl], start=False, stop=True)
                                opsT.append(ops)
                            dsb = rowa_p.tile([1, 2, NTOK], F32, tag="ds")
                            for j in range(2):
                                nc.scalar.activation(out=dsb[0:1, j, 0:tl],
                                                     in_=opsT[j][64:65, 0:tl],
                                                     func=AF.Copy)
                            rpair = rowa_p.tile([1, 2, NTOK], F32, tag="rr")
                            nc.vector.reciprocal_approx_fast(
                                out=rpair[0:1, :, 0:tl], in_=dsb[0:1, :, 0:tl])
                            rpairB = rowa_p.tile([1, 2, NTOK], BF16, tag="rb")
                            if gpsa:
                                for j in range(2):
                                    h = 2 * ch + j
                                    nc.vector.tensor_scalar(
                                        out=rpairB[0:1, j, 0:tl], in0=rpair[0:1, j, 0:tl],
                                        scalar1=pos_ctx["omsig"][0:1, h:h + 1],
                                        scalar2=None, op0=ALU.mult)
                            else:
                                nc.vector.tensor_copy(out=rpairB[:, :, 0:tl],
                                                      in_=rpair[:, :, 0:tl])
                            cps = aps.tile([128, 2, NTOK], F32, tag="cr")
                            nc.tensor.matmul(cps[:, :, 0:tl], ones_row,
                                             rpairB[:, :, 0:tl], start=True, stop=True)
                            c1R = tmp1_p.tile([128, 2, NTOK], F32, tag="c1R")
                            nc.scalar.activation(out=c1R[:, :, 0:tl], in_=cps[:, :, 0:tl],
                                                 func=AF.Copy)
                            if gpsa:
                                fps = aps.tile([128, NTOK], F32, tag="of")
                                FT = pos_ctx["FT"]
                                tf = tmp1_p.tile([128, NTOK], F32, tag="tf")
                                for j in range(2):
                                    h = 2 * ch + j
                                    off = j * 64
                                    nc.tensor.matmul(fps[off:off + 64, 0:tl],
                                                     vt[(bi, 0)][:, h, 0:64],
                                                     FT[0][:, h, :], start=True, stop=False)
                                    nc.tensor.matmul(fps[off:off + 64, 0:tl],
                                                     vt[(bi, 1)][:kl, h, 0:64],
                                                     FT[1][:kl, h, :], start=False, stop=True)
                                    nc.vector.tensor_tensor(
                                        out=tf[off:off + 64, 0:tl],
                                        in0=opsT[j][0:64, 0:tl],
                                        in1=c1R[0:64, j, 0:tl], op=ALU.mult)
                                for j in range(2):
                                    h = 2 * ch + j
                                    off = j * 64
                                    nc.vector.scalar_tensor_tensor(
                                        out=OT[ch][off:off + 64, b, t0:t0 + tl],
                                        in0=fps[off:off + 64, 0:tl],
                                        scalar=pos_ctx["g128"][off:off + 64, h:h + 1],
                                        in1=tf[off:off + 64, 0:tl],
                                        op0=ALU.mult, op1=ALU.add)
                            else:
                                for j in range(2):
                                    off = j * 64
                                    nc.vector.tensor_tensor(
                                        out=OT[ch][off:off + 64, b, t0:t0 + tl],
                                        in0=opsT[j][0:64, 0:tl],
                                        in1=c1R[0:64, j, 0:tl], op=ALU.mult)

        def transformer_layer(L, li, gpsa):
            t0, tl = (1, NPATCH) if gpsa else (0, NTOK)
            n1s = load_sm(L["n1s"], NCH, "n1s")
            n1b = load_sm(L["n1b"], NCH, "n1b")
            n2s = load_sm(L["n2s"], NCH, "n2s")
            n2b = load_sm(L["n2b"], NCH, "n2b")
            projb_sb = load_sm(L["projb"], NCH, "projb")

            xn = [act_p.tile([128, B_CORE, NTOK], BF16, tag=f"act{c}", name=f"xn{c}") for c in range(NCH)]
            pos_ctx = {}
            if gpsa:
                pos_ctx["w_qk"] = load_wT(L["qkT"], 2 * C, "wbig")
                v_sb = load_wT(L["vT"], C, "wbig2")
                pos_ctx["w_v"] = lambda kc: v_sb[:, kc, :]
                # pos softmax per head (token-major), then transpose to F^T
                poswR = wsm_p.tile([128, H, 3], F32, tag="poswR")
                pw_src = L["posw"]
                nc.sync.dma_start(out=poswR, in_=bass.AP(
                    tensor=pw_src.tensor, offset=pw_src.offset,
                    ap=[[0, 128]] + [list(a) for a in pw_src.ap]))
                gate_sb = wsm_p.tile([1, H], F32, tag="gate")
                nc.sync.dma_start(out=gate_sb, in_=L["gate"])
                sig = wsm_p.tile([1, H], F32, tag="sig")
                nc.scalar.activation(out=sig, in_=gate_sb, func=AF.Sigmoid)
                sigb = wsm_p.tile([1, H], BF16, tag="sigb")
                nc.vector.tensor_copy(out=sigb, in_=sig)
                omsig = wsm_p.tile([1, H], F32, tag="omsig")
                nc.vector.tensor_scalar(out=omsig, in0=sig, scalar1=-1.0, scalar2=1.0,
                                        op0=ALU.mult, op1=ALU.add)
                pos_ctx["omsig"] = omsig
                FT = [ft_p.tile([128, H, NPATCH], BF16, tag=f"ft{i}", name=f"ft{i}") for i in range(2)]
                pos_ctx["FT"] = FT
                g128 = wsm_p.tile([128, H], F32, tag="g64")
                pos_ctx["g128"] = g128
                nhalf = [(0, 128), (128, 68)]
                with tc.tile_pool(name=f"gpos{li}", bufs=2, space="PSUM") as pps:
                    g128_ps = pps.tile([128, H], F32, tag="tr")
                    nc.tensor.matmul(g128_ps, ones_row, sigb, start=True, stop=True)
                    nc.vector.tensor_copy(out=g128, in_=g128_ps)
                    for h in range(H):
                        for (n0, nl) in nhalf:
                            nh = 0 if n0 == 0 else 1
                            t = tmp1_p.tile([128, NPATCH], F32, tag="lt")
                            nc.vector.tensor_scalar(
                                out=t[:nl], in0=rel_sb[nh][:nl, 0, :],
                                scalar1=poswR[:nl, h, 0:1], scalar2=None, op0=ALU.mult)
                            nc.vector.scalar_tensor_tensor(
                                out=t[:nl], in0=rel_sb[nh][:nl, 1, :],
                                scalar=poswR[:nl, h, 1:2], in1=t[:nl],
                                op0=ALU.mult, op1=ALU.add)
                            nc.vector.scalar_tensor_tensor(
                                out=t[:nl], in0=rel_sb[nh][:nl, 2, :],
                                scalar=poswR[:nl, h, 2:3], in1=t[:nl],
                                op0=ALU.mult, op1=ALU.add)
                            nmax = row_p.tile([128, 1], F32, tag="nmax")
                            nc.vector.reduce_max(out=nmax[:nl], in_=t[:nl],
                                                 axis=mybir.AxisListType.X, negate=True)
                            fe = tmp1_p.tile([128, NPATCH], F32, tag="ltb")
                            fsum = row_p.tile([128, 1], F32, tag="fsum")
                            nc.scalar.activation(out=fe[:nl], in_=t[:nl], func=AF.Exp,
                                                 bias=nmax[:nl], scale=1.0,
                                                 accum_out=fsum[:nl])
                            nc.vector.reciprocal(out=fsum[:nl], in_=fsum[:nl])
                            fn = tmp1_p.tile([128, NPATCH], BF16, tag="tf")
                            nc.vector.tensor_scalar(out=fn[:nl], in0=fe[:nl],
                                                    scalar1=fsum[:nl], scalar2=None,
                                                    op0=ALU.mult)
                            for (m0, mlen) in nhalf:
                                mh = 0 if m0 == 0 else 1
                                trp = pps.tile([128, 128], BF16, tag="tr")
                                nc.tensor.transpose(trp[:mlen, :nl], fn[:nl, m0:m0 + mlen],
                                                    ident[:nl, :nl])
                                nc.vector.tensor_copy(out=FT[mh][:mlen, h, n0:n0 + nl],
                                                      in_=trp[:mlen, :nl])
            else:
                qkv_sb = load_wT(L["qkvT"], 3 * C, "wbig")
                pos_ctx["w_qk"] = qkv_sb
                pos_ctx["w_v"] = lambda kc: qkv_sb[:, kc, 2 * C:3 * C]

            with tc.tile_pool(name=f"ln1s{li}_{gpsa}", bufs=2, space="PSUM") as stp, \
                 tc.tile_pool(name=f"ln1m{li}_{gpsa}", bufs=4, space="PSUM") as mmp:
                ln(xn, n1s, n1b, t0, tl, stp, mmp)

            OT = [act_p.tile([128, B_CORE, NTOK], BF16, tag=f"act{c}", name=f"ot{c}") for c in range(NCH)]
            qkv_and_attention(L, xn, OT, t0, tl, gpsa, pos_ctx)

            proj_sb = load_wT(L["projT"], C, "wbig2")
            with tc.tile_pool(name=f"proj{li}_{gpsa}", bufs=6, space="PSUM") as mmp:
                add_linear(proj_sb, projb_sb, t0, tl,
                           lambda kc, b0: OT[kc][:, b0:b0 + 2, t0:t0 + tl], mmp)

            xn2 = [act_p.tile([128, B_CORE, NTOK], BF16, tag=f"act{c}", name=f"xn2_{c}") for c in range(NCH)]
            with tc.tile_pool(name=f"ln2s{li}_{gpsa}", bufs=2, space="PSUM") as stp, \
                 tc.tile_pool(name=f"mlp{li}_{gpsa}", bufs=4, space="PSUM") as mmp:
                ln(xn2, n2s, n2b, t0, tl, stp, mmp)
                mlp(L, xn2, t0, tl, stp, mmp)

        for li, L in enumerate(gl):
            transformer_layer(L, li, True)
        for li, L in enumerate(ml):
            transformer_layer(L, li, False)

        # ---- final LN on cls + head -------------------------------------
        fns_sb = load_sm(fns, NCH, "n1s")
        fnb_sb = load_sm(fnb, NCH, "n1b")
        hw_sb = w_p.tile([128, NCH, 1024], BF16, tag="wbig")
        nc.sync.dma_start(out=hw_sb, in_=headT.rearrange("(c p) n -> p c n", p=128))
        hb_sb = load_sm(head_b, 8, "fc1b")

        with tc.tile_pool(name="fin0", bufs=1, space="PSUM") as stp, \
             tc.tile_pool(name="fin", bufs=4, space="PSUM") as mmp:
            s_ps = stp.tile([1, B_CORE], F32, tag="st1")
            q_ps = stp.tile([1, B_CORE], F32, tag="st2")
            for c in range(NCH):
                xb = tmp_p.tile([128, B_CORE], BF16, tag="fxb")
                xq = tmp_p.tile([128, B_CORE], BF16, tag="fxq")
                sl = res[c][:, :, 0]
                nc.vector.tensor_copy(out=xb, in_=sl)
                nc.vector.tensor_tensor(out=xq, in0=sl, in1=sl, op=ALU.mult)
                nc.tensor.matmul(s_ps, ones_col, xb, start=(c == 0), stop=(c == NCH - 1))
                nc.tensor.matmul(q_ps, ones_col, xq, start=(c == 0), stop=(c == NCH - 1))
            mean = row_p.tile([1, B_CORE], F32, tag="r1")
            var = row_p.tile([1, B_CORE], F32, tag="r2")
            msq = row_p.tile([1, B_CORE], F32, tag="r3")
            nc.vector.tensor_scalar_mul(mean, s_ps, 1.0 / C)
            nc.vector.tensor_scalar_mul(var, q_ps, 1.0 / C)
            nc.vector.tensor_tensor(out=msq, in0=mean, in1=mean, op=ALU.mult)
            nc.vector.tensor_sub(var, var, msq)
            nc.scalar.activation(out=var, in_=var, func=AF.Sqrt, bias=eps_sb[0:1, :], scale=1.0)
            nc.vector.reciprocal(out=var, in_=var)
            meanb = row_p.tile([1, B_CORE], BF16, tag="r4")
            rstdb = row_p.tile([1, B_CORE], BF16, tag="r5")
            nc.vector.tensor_copy(out=meanb, in_=mean)
            nc.vector.tensor_copy(out=rstdb, in_=var)
            mR = mmp.tile([128, B_CORE], F32, tag="mm")
            rR = mmp.tile([128, B_CORE], F32, tag="mm")
            nc.tensor.matmul(mR, ones_row, meanb, start=True, stop=True)
            nc.tensor.matmul(rR, ones_row, rstdb, start=True, stop=True)
            xnf = []
            for c in range(NCH):
                t = tmp_p.tile([128, B_CORE], F32, tag="flt")
                nc.vector.tensor_sub(t, res[c][:, :, 0], mR)
                nc.vector.tensor_tensor(out=t, in0=t, in1=rR, op=ALU.mult)
                xc = tmp_p.tile([128, B_CORE], BF16, tag=f"fxn{c}")
                nc.vector.tensor_scalar(out=xc, in0=t, scalar1=fns_sb[:, c:c + 1],
                                        scalar2=fnb_sb[:, c:c + 1],
                                        op0=ALU.mult, op1=ALU.add)
                xnf.append(xc)
            for mc in range(8):
                ps = mmp.tile([128, B_CORE], F32, tag="mm")
                for kc in range(NCH):
                    nc.tensor.matmul(ps, hw_sb[:, kc, mc * 128:(mc + 1) * 128], xnf[kc],
                                     start=(kc == 0), stop=(kc == NCH - 1))
                ot = tmp_p.tile([128, B_CORE], F32, tag="fout")
                nc.vector.tensor_scalar(out=ot, in0=ps, scalar1=hb_sb[:, mc:mc + 1],
                                        scalar2=None, op0=ALU.add)
                nc.sync.dma_start(out=OUT[mc * 128:(mc + 1) * 128, :], in_=ot)

    nc.compile()
    return nc


# ---------------------------------------------------------------------------
# host side
# ---------------------------------------------------------------------------

def _rel_nkm():
    ind = np.arange(GRID)[None, :] - np.arange(GRID)[:, None]
    indx = np.tile(ind, (GRID, GRID)).astype(np.float32)
    indy = np.repeat(np.repeat(ind, GRID, axis=0), GRID, axis=1).astype(np.float32)
    indd = indx ** 2 + indy ** 2
    rel = np.stack([indx, indy, indd], axis=0)           # [3, n, m]
    return np.ascontiguousarray(rel.transpose(1, 0, 2)).astype(np.float16)  # [n, 3, m]


def _pcol(v, parts=128):
    v = np.asarray(v, np.float32).reshape(-1, parts)
    return np.ascontiguousarray(v.T)


def _wT(w):
    return np.ascontiguousarray(np.asarray(w, np.float32).T.astype(BF))


def _prep_weights(i):
    d = {}
    d["pwT"] = _wT(np.asarray(i["patch_w"], np.float32).reshape(C, C))
    d["pb"] = _pcol(i["patch_b"])
    d["posT"] = np.ascontiguousarray(np.asarray(i["pos_embed"], np.float32)[0].T)
    d["cls"] = np.asarray(i["cls_token"], np.float32).reshape(C)
    d["rel"] = _rel_nkm()
    hT = np.zeros((C, 1024), np.float32)
    hT[:, :1000] = np.asarray(i["head_w"], np.float32).T
    d["headT"] = np.ascontiguousarray(hT.astype(BF))
    hb = np.zeros(1024, np.float32)
    hb[:1000] = np.asarray(i["head_b"], np.float32)
    d["head_b"] = _pcol(hb)
    d["fns"] = _pcol(i["norm_s"])
    d["fnb"] = _pcol(i["norm_b"])
    for l in range(L_G):
        d[f"g{l}_qkT"] = _wT(i["g_qk_w"][l])
        d[f"g{l}_vT"] = _wT(i["g_v_w"][l])
        d[f"g{l}_projT"] = _wT(i["g_proj_w"][l])
        d[f"g{l}_projb"] = _pcol(i["g_proj_b"][l])
        d[f"g{l}_fc1T"] = _wT(i["g_fc1_w"][l])
        d[f"g{l}_fc1b"] = _pcol(i["g_fc1_b"][l])
        d[f"g{l}_fc2T"] = _wT(i["g_fc2_w"][l])
        d[f"g{l}_fc2b"] = _pcol(i["g_fc2_b"][l])
        d[f"g{l}_n1s"] = _pcol(i["g_norm1_s"][l])
        d[f"g{l}_n1b"] = _pcol(i["g_norm1_b"][l])
        d[f"g{l}_n2s"] = _pcol(i["g_norm2_s"][l])
        d[f"g{l}_n2b"] = _pcol(i["g_norm2_b"][l])
        d[f"g{l}_posw"] = np.ascontiguousarray(np.asarray(i["g_pos_w"][l], np.float32))
        d[f"g{l}_gate"] = np.ascontiguousarray(
            np.asarray(i["g_gate"][l], np.float32).reshape(1, H))
    for l in range(L_M):
        d[f"m{l}_qkvT"] = _wT(i["m_qkv_w"][l])
        d[f"m{l}_projT"] = _wT(i["m_proj_w"][l])
        d[f"m{l}_projb"] = _pcol(i["m_proj_b"][l])
        d[f"m{l}_fc1T"] = _wT(i["m_fc1_w"][l])
        d[f"m{l}_fc1b"] = _pcol(i["m_fc1_b"][l])
        d[f"m{l}_fc2T"] = _wT(i["m_fc2_w"][l])
        d[f"m{l}_fc2b"] = _pcol(i["m_fc2_b"][l])
        d[f"m{l}_n1s"] = _pcol(i["m_norm1_s"][l])
        d[f"m{l}_n1b"] = _pcol(i["m_norm1_b"][l])
        d[f"m{l}_n2s"] = _pcol(i["m_norm2_s"][l])
        d[f"m{l}_n2b"] = _pcol(i["m_norm2_b"][l])
    return d


_last_results = None


def build_in_maps(inputs):
    wmap = _prep_weights(inputs)
    x = np.asarray(inputs["x"], np.float32)
    in_maps = []
    for core in range(NCORES):
        xs = x[core * B_CORE:(core + 1) * B_CORE]
        xi = xs.reshape(B_CORE, 3, GRID, PS, GRID, PS).transpose(1, 3, 5, 0, 2, 4)
        xi = np.ascontiguousarray(xi.reshape(C, B_CORE * NPATCH).astype(BF))
        m = dict(wmap)
        m["xim"] = xi
        in_maps.append(m)
    return in_maps


def get_program():
    if "nc" not in _CACHE:
        _CACHE["nc"] = _build_program()
    return _CACHE["nc"]


def kernel(**inputs):
    global _last_results
    _install_ntff_hook()
    from concourse import bass_utils

    nc = get_program()
    in_maps = build_in_maps(inputs)
    res = bass_utils.run_bass_kernel_spmd(nc, in_maps, core_ids=list(range(NCORES)))
    _last_results = res
    outs = [r["out"][:1000, :].T for r in res.results]
    return np.ascontiguousarray(np.concatenate(outs, axis=0).astype(np.float32))

